# revision 20
# baseline (speedup 1.0000x reference)
"""Trainium2 Bass kernel for the A2GNN 2-layer attention GNN (N=8192, d=128).

Row-parallel over 8 NeuronCores: core r owns rows [r*1024, (r+1)*1024).

Math restructuring (verified exact vs the reference on its data distribution):
  h0 = relu(x @ w_embed) >= 0 elementwise, and softmax aggregation keeps
  h >= 0, so cos(h_i, h_j) >= 0 always and the (cos < 0) mask never fires.
  With NEG = -1e9, exp(mask) collapses to a multiplicative factor:
      E1 = E2 * exp(beta*cos),   E2 = aff * (1 + (e^10 - 1)*adj)
  (layer 2 has beta = 0, so P2 = E2 / rowsum(E2) -- no score matmul at all).

Distribution choices (host-side shard prep):
  - E2^T is precomputed per-shard in bf16 (exact: entries are {0,1,e^10})
    and streamed straight into a resident SBUF tensor, read by both layers.
  - x^T is replicated in bf16 so every core embeds the FULL h0 locally --
    no first AllGather; the only collective is the tiny h1 gather.

Everything runs in the transposed [j, i] orientation: scores^T from
qT/qTloc matmuls (q built by a transposed embed h0^T = w^T x^T with norms
from an ones-column matmul and WIDE broadcast-then-reciprocal -- [1,N]
single-lane vector ops are ~6x slower than [128,N] ones), aggregation as
lhsT = h-tile, rhs = E1^T with N=512 moving tensors, rowsums from M=1 ones
matmuls.  Layer-2's s2 = colsum(E2^T) matmuls are issued right after the
AllGather trigger so the PE fills the collective latency; out = h2 @ w_out
is computed at the very end from h2^T (no z-carry through the gather).
The embed chunks and layer-1 j-tiles run as one software pipeline.
"""

import sys

import numpy as np

N = 8192
NCORES = 8
R = N // NCORES          # 1024 rows per core
D = 128                  # hidden dim
NF = 512                 # input features
NCLS = 21                # classes
NJ = N // 128            # 64 j-tiles of 128
NCH = N // 512           # 16 embed chunks of 512
KF = NF // 128           # 4 k-tiles for the embed matmul
KEXP = float(np.exp(10.0) - 1.0)


def _ensure_concourse():
    try:
        import concourse.bass  # noqa: F401
    except ImportError:
        sys.path.insert(0, "/opt/trn_rl_repo")


def _fix_sem_waits(nc, mybir, max_waits=1):
    """This container's walrus accepts at most 1 sem-wait per instruction.
    Keep the first wait on each instruction; move the rest onto preceding
    same-engine NoOps (the engine stalls there first, so semantics are
    preserved).  Ctrl-type instructions (Drain/NoOp) get all waits moved."""
    n_fixed = 0
    for bb in nc.main_func.blocks:
        insts = bb.instructions
        if not any(
            i.sync_info is not None
            and i.sync_info.on_wait
            and len(i.sync_info.on_wait) > max_waits
            for i in insts
        ):
            continue
        out = []
        for ins in insts:
            si = ins.sync_info
            if si is not None and si.on_wait and len(si.on_wait) > max_waits:
                waits = list(si.on_wait)
                is_ctrl = type(ins).__name__ in ("InstDrain", "InstNoOp")
                keep = [] if is_ctrl else waits[:max_waits]
                spill = waits if is_ctrl else waits[max_waits:]
                for k in range(0, len(spill), max_waits):
                    out.append(
                        mybir.InstNoOp(
                            name=f"{ins.name}-dw{k}",
                            engine=ins.engine,
                            bass_nofuse=True,
                            sync_info=mybir.SyncInfo(
                                on_wait=spill[k : k + max_waits], on_update=[]
                            ),
                        )
                    )
                ins.sync_info = mybir.SyncInfo(on_wait=keep, on_update=list(si.on_update))
                n_fixed += 1
            out.append(ins)
        insts.clear()
        insts.extend(out)
    return n_fixed


def build_nc():
    _ensure_concourse()
    import concourse.bass as bass
    import concourse.mybir as mybir
    import concourse.tile as tile
    from concourse import masks

    F32 = mybir.dt.float32
    BF16 = mybir.dt.bfloat16
    AF = mybir.ActivationFunctionType
    core_ids = list(range(NCORES))

    nc = bass.Bass()
    xtf_ext = nc.declare_dram_parameter("xTfull", [NF, N], BF16, isOutput=False)
    xtl_ext = nc.declare_dram_parameter("xTloc", [NF, R], BF16, isOutput=False)
    e2t_ext = nc.declare_dram_parameter("E2T", [N, R], BF16, isOutput=False)
    s2_ext = nc.declare_dram_parameter("s2loc", [1, R], BF16, isOutput=False)
    wemb_ext = nc.declare_dram_parameter("w_embed", [NF, D], F32, isOutput=False)
    wout_ext = nc.declare_dram_parameter("w_out", [D, NCLS], F32, isOutput=False)
    beta_ext = nc.declare_dram_parameter("beta", [1], F32, isOutput=False)
    fts_ext = nc.declare_dram_parameter("fts", [R, D], F32, isOutput=True)
    out_ext = nc.declare_dram_parameter("out", [R, NCLS], F32, isOutput=True)

    with tile.TileContext(nc) as tc:
        with (
            tc.tile_pool(name="persist", bufs=1) as persist,
            tc.tile_pool(name="dram", bufs=1, space="DRAM") as dram,
            tc.tile_pool(name="srs_ps", bufs=2, space="PSUM") as srs_ps,
        ):
            E2T = persist.tile([128, NJ, R], BF16)       # resident masked-exp^T
            qTlocb = persist.tile([128, R], BF16)        # beta * q^T local slice
            wemb_b = persist.tile([128, KF, D], BF16)
            wout_sb = persist.tile([128, NCLS], BF16)
            betab = persist.tile([128, 1], F32)
            ident = persist.tile([128, 128], F32)
            identb = persist.tile([128, 128], BF16)
            ones1 = persist.tile([1, 128], F32)          # K=1 broadcast weights
            ones1b = persist.tile([1, 128], BF16)
            onescolb = persist.tile([128, 1], BF16)      # M=1 column-sum weights

            masks.make_identity(nc, ident[:])
            masks.make_identity(nc, identb[:])
            nc.vector.memset(ones1[:], 1.0)
            nc.vector.memset(ones1b[:], 1.0)
            nc.vector.memset(onescolb[:], 1.0)

            b2h = [None, None]
            ag2h = [None, None]
            for hh in range(2):
                b2h[hh] = dram.tile([R // 2, D], BF16, name=f"b2h{hh}")
                ag2h[hh] = dram.tile([N // 2, D], BF16, addr_space="Shared", name=f"ag2h{hh}")

            with (
                tc.tile_pool(name="ph1", bufs=1) as ph1,
                tc.tile_pool(name="pa", bufs=3) as pa,
                tc.tile_pool(name="pe", bufs=3) as pe,
                tc.tile_pool(name="aux_ps", bufs=2, space="PSUM") as aux_ps,
                tc.tile_pool(name="sc_ps", bufs=2, space="PSUM") as sc_ps,
                tc.tile_pool(name="agg_ps", bufs=2, space="PSUM") as agg_ps,
            ):
                qT = ph1.tile([128, N], BF16)
                haug1 = ph1.tile([128, NJ, D + 1], BF16)
                nc.vector.memset(haug1[:, :, D : D + 1], 1.0)

                # -------- weights / beta --------
                wtmp = pa.tile([128, KF, D], F32, bufs=1)
                nc.sync.dma_start(
                    wtmp[:], wemb_ext[:].rearrange("(k p) d -> p k d", p=128)
                )
                nc.vector.tensor_copy(wemb_b[:], wtmp[:])
                wotmp = pa.tile([128, NCLS], F32, bufs=1)
                nc.sync.dma_start(wotmp[:], wout_ext[:])
                nc.vector.tensor_copy(wout_sb[:], wotmp[:])
                btmp = pa.tile([1, 1], F32, bufs=1)
                nc.sync.dma_start(btmp[:], beta_ext[None, :])
                ps_b = aux_ps.tile([128, 1], F32, tag="aux")
                nc.tensor.matmul(ps_b[:], ones1[:], btmp[:], start=True, stop=True)
                nc.scalar.copy(betab[:], ps_b[:])

                n2T = ph1.tile([128, NJ], F32)       # per-j squared norms
                rinvjb = ph1.tile([128, NJ], F32)    # beta / |h_j| (exp scales)

                def embed_raw(src_ext, c0, dst_qT, dst_off):
                    """dst_qT[:, cols] = relu(w^T x^T) (bf16, unnormalized);
                    returns the bf16 squared tile for norm accumulation."""
                    xk = pa.tile([128, KF, 512], BF16, tag="xk", bufs=2)
                    for kt in range(KF):
                        nc.sync.dma_start(
                            xk[:, kt, :],
                            src_ext[kt * 128 : (kt + 1) * 128, c0 : c0 + 512],
                        )
                    ps_hT = sc_ps.tile([128, 512], F32, tag="sc")
                    for kt in range(KF):
                        nc.tensor.matmul(
                            ps_hT[:],
                            wemb_b[:, kt, :],
                            xk[:, kt, :],
                            start=(kt == 0),
                            stop=(kt == KF - 1),
                        )
                    hslice = dst_qT[:, dst_off : dst_off + 512]
                    nc.scalar.activation(hslice, ps_hT[:], AF.Relu)
                    sq = pa.tile([128, 512], BF16, tag="sqt", bufs=2)
                    nc.vector.tensor_mul(sq[:], hslice, hslice)
                    return sq

                # local slice first (normalized + beta-scaled, unblocks scores)
                rbeta = pa.tile([128, 1], F32, bufs=1)
                nc.vector.reciprocal(rbeta[:], betab[:])
                for ch in range(2):
                    sq = embed_raw(xtl_ext, ch * 512, qTlocb, ch * 512)
                    ps_n = aux_ps.tile([1, 512], F32, tag="aux")
                    nc.tensor.matmul(ps_n[:], onescolb[:], sq[:], start=True, stop=True)
                    nrm = pa.tile([1, 512], F32, tag="nrm")
                    nc.scalar.sqrt(nrm[:], ps_n[:])
                    nc.vector.tensor_scalar_mul(nrm[:], nrm[:], rbeta[0:1, 0:1])
                    nrmb = pa.tile([1, 512], BF16, tag="nrmb")
                    nc.vector.tensor_copy(nrmb[:], nrm[:])
                    ps_bc = aux_ps.tile([128, 512], F32, tag="aux")
                    nc.tensor.matmul(ps_bc[:], ones1b[:], nrmb[:], start=True, stop=True)
                    rcp = pa.tile([128, 512], F32, tag="rcpw", bufs=2)
                    nc.vector.reciprocal(rcp[:], ps_bc[:])
                    rcpb = pa.tile([128, 512], BF16, tag="rcpwb", bufs=2)
                    nc.vector.tensor_copy(rcpb[:], rcp[:])
                    nc.vector.tensor_mul(
                        qTlocb[:, ch * 512 : (ch + 1) * 512],
                        qTlocb[:, ch * 512 : (ch + 1) * 512],
                        rcpb[:],
                    )

                agg1 = [agg_ps.tile([128, 512], F32, tag="agg", name=f"agg1_{i}") for i in range(2)]
                srs1 = [srs_ps.tile([1, 512], F32, tag="srs", name=f"srs1_{i}") for i in range(2)]

                def prep_chunk(ch):
                    """E2T stream + embed + per-j norms + haug1 transposes."""
                    nc.sync.dma_start(
                        E2T[:, ch * 4 : (ch + 1) * 4, :],
                        e2t_ext[ch * 512 : (ch + 1) * 512, :].rearrange(
                            "(t p) i -> p t i", p=128
                        ),
                    )
                    sq = embed_raw(xtf_ext, ch * 512, qT, ch * 512)
                    for q in range(4):
                        jt = ch * 4 + q
                        ps_nj = aux_ps.tile([128, 1], F32, tag="aux")
                        nc.tensor.matmul(
                            ps_nj[:],
                            sq[:, q * 128 : (q + 1) * 128],
                            onescolb[:],
                            start=True,
                            stop=True,
                        )
                        nc.vector.tensor_copy(n2T[:, jt : jt + 1], ps_nj[:])
                        ps_t = aux_ps.tile([128, 128], BF16, tag="aux")
                        nc.tensor.transpose(
                            ps_t[:], qT[:, jt * 128 : (jt + 1) * 128], identb[:]
                        )
                        nc.vector.tensor_copy(haug1[:, jt, 0:D], ps_t[:])

                def layer1_jt(jt):
                    for bq in range(2):
                        ps_sc = sc_ps.tile([128, 512], F32, tag="sc")
                        nc.tensor.matmul(
                            ps_sc[:],
                            qT[:, jt * 128 : (jt + 1) * 128],
                            qTlocb[:, bq * 512 : (bq + 1) * 512],
                            start=True,
                            stop=True,
                        )
                        e1t = pe.tile([128, 512], BF16, tag="e1t", bufs=3)
                        nc.scalar.activation(
                            e1t[:], ps_sc[:], AF.Exp, scale=rinvjb[:, jt : jt + 1]
                        )
                        nc.vector.tensor_mul(
                            e1t[:], E2T[:, jt, bq * 512 : (bq + 1) * 512], e1t[:]
                        )
                        nc.tensor.matmul(
                            agg1[bq][:],
                            haug1[:, jt, 0:D],
                            e1t[:],
                            start=(jt == 0),
                            stop=(jt == NJ - 1),
                        )
                        nc.tensor.matmul(
                            srs1[bq][:],
                            haug1[:, jt, D : D + 1],
                            e1t[:],
                            start=(jt == 0),
                            stop=(jt == NJ - 1),
                        )

                def finish_norms(c0, cn):
                    nrmj = pa.tile([128, cn], F32, tag="nrmj", bufs=2)
                    nc.scalar.sqrt(nrmj[:], n2T[:, c0 : c0 + cn])
                    nc.vector.reciprocal(
                        rinvjb[:, c0 : c0 + cn], nrmj[:]
                    )
                    nc.vector.tensor_scalar_mul(
                        rinvjb[:, c0 : c0 + cn], rinvjb[:, c0 : c0 + cn], betab[:, 0:1]
                    )

                # block pipeline: prep 4 chunks (16 jts), batch the norm
                # sqrt (avoids ACT Sqrt<->Exp table reloads), trail layer-1
                # by one block
                for blk in range(4):
                    for i in range(4):
                        prep_chunk(blk * 4 + i)
                        if blk >= 1:
                            for q in range(4):
                                layer1_jt((blk - 1) * 16 + i * 4 + q)
                    finish_norms(blk * 16, 16)
                for i in range(4):
                    for q in range(4):
                        layer1_jt(48 + i * 4 + q)

                # ----- layer-1 finish: normalize, bounce, AllGather -----
                for bq in range(2):
                    srsb = pe.tile([1, 512], BF16, tag="srsb", bufs=2)
                    nc.scalar.copy(srsb[:], srs1[bq][:])
                    bcs_ps = aux_ps.tile([128, 512], F32, tag="aux")
                    nc.tensor.matmul(bcs_ps[:], ones1b[:], srsb[:], start=True, stop=True)
                    rcp1 = pe.tile([128, 512], F32, tag="rcp1", bufs=1)
                    nc.vector.reciprocal(rcp1[:], bcs_ps[:])
                    h1T = pe.tile([128, 512], BF16, tag="h1T", bufs=1)
                    nc.vector.tensor_mul(h1T[:], agg1[bq][:], rcp1[:])
                    for q in range(4):
                        r0 = (bq * 4 + q) * 128
                        ps_t = aux_ps.tile([128, 128], BF16, tag="aux")
                        nc.tensor.transpose(
                            ps_t[:], h1T[:, q * 128 : (q + 1) * 128], identb[:]
                        )
                        h1row = pe.tile([128, 128], BF16, tag="h1row", bufs=2)
                        nc.vector.tensor_copy(h1row[:], ps_t[:])
                        nc.scalar.dma_start(
                            b2h[bq][q * 128 : (q + 1) * 128, :], h1row[:]
                        )
                    # gather this half while the other half finishes
                    nc.gpsimd.collective_compute(
                        "AllGather",
                        mybir.AluOpType.bypass,
                        ins=[b2h[bq][:]],
                        outs=[ag2h[bq][:]],
                        replica_groups=[core_ids],
                    )

                # s2 = rowsums of E2 come precomputed from the host
                s2sb = persist.tile([1, R], BF16)
                nc.scalar.dma_start(s2sb[:], s2_ext[:])

            # ===== layer 2: h2^T = (E2 @ h1)^T / s2; out = h2 @ w_out =====
            with (
                tc.tile_pool(name="pf", bufs=3) as pf,
                tc.tile_pool(name="sc2_ps", bufs=2, space="PSUM") as sc2_ps,
                tc.tile_pool(name="agg2_ps", bufs=2, space="PSUM") as agg2_ps,
            ):
                haug2 = pf.tile([128, NJ, D], BF16, bufs=1)
                for hh in range(2):
                    for r in range(NCORES):
                        nc.scalar.dma_start(
                            haug2[:, 8 * r + 4 * hh : 8 * r + 4 * hh + 4, :],
                            ag2h[hh][r * 512 : (r + 1) * 512, :].rearrange(
                                "(t p) c -> p t c", p=128
                            ),
                        )

                agg2 = [agg2_ps.tile([128, 512], F32, tag="agg2", name=f"agg2_{i}") for i in range(2)]
                for jt in range(NJ):
                    for bq in range(2):
                        nc.tensor.matmul(
                            agg2[bq][:],
                            haug2[:, jt, :],
                            E2T[:, jt, bq * 512 : (bq + 1) * 512],
                            start=(jt == 0),
                            stop=(jt == NJ - 1),
                        )

                for bq in range(2):
                    bc2_ps = sc2_ps.tile([128, 512], F32, tag="sc2")
                    nc.tensor.matmul(
                        bc2_ps[:], ones1b[:], s2sb[0:1, bq * 512 : (bq + 1) * 512],
                        start=True, stop=True,
                    )
                    rcp2 = pf.tile([128, 512], F32, tag="rcp2", bufs=2)
                    nc.vector.reciprocal(rcp2[:], bc2_ps[:])
                    h2T = pf.tile([128, 512], BF16, tag="h2T", bufs=2)
                    nc.vector.tensor_mul(h2T[:], agg2[bq][:], rcp2[:])
                    for q in range(4):
                        r0 = (bq * 4 + q) * 128
                        ps_ft = sc2_ps.tile([128, 128], BF16, tag="sc2")
                        nc.tensor.transpose(
                            ps_ft[:], h2T[:, q * 128 : (q + 1) * 128], identb[:]
                        )
                        ftsrow = pf.tile([128, 128], F32, tag="ftsrow", bufs=2)
                        nc.scalar.copy(ftsrow[:], ps_ft[:])
                        nc.sync.dma_start(fts_ext[r0 : r0 + 128, :], ftsrow[:])
                        ps_o = sc2_ps.tile([128, NCLS], F32, tag="sc2")
                        nc.tensor.matmul(
                            ps_o[:],
                            h2T[:, q * 128 : (q + 1) * 128],
                            wout_sb[:],
                            start=True,
                            stop=True,
                        )
                        outrow = pf.tile([128, NCLS], F32, tag="outrow", bufs=2)
                        nc.scalar.copy(outrow[:], ps_o[:])
                        nc.sync.dma_start(out_ext[r0 : r0 + 128, :], outrow[:])

    _fix_sem_waits(nc, __import__("concourse.mybir", fromlist=["mybir"]))
    return nc


def make_in_maps(x, adj, aff_cropping, w_embed, w_out, beta):
    import ml_dtypes

    bf16 = ml_dtypes.bfloat16
    x = np.asarray(x, dtype=np.float32)
    adj = np.asarray(adj, dtype=np.float32)
    aff = np.asarray(aff_cropping, dtype=np.float32)
    w_embed = np.ascontiguousarray(np.asarray(w_embed, dtype=np.float32))
    w_out = np.ascontiguousarray(np.asarray(w_out, dtype=np.float32))
    beta = np.ascontiguousarray(np.asarray(beta, dtype=np.float32))

    xTfull = np.ascontiguousarray(x.T.astype(bf16))
    in_maps = []
    for r in range(NCORES):
        sl = slice(r * R, (r + 1) * R)
        e2 = aff[sl] * (1.0 + KEXP * adj[sl])
        e2b = e2.astype(bf16)
        in_maps.append(
            {
                "xTfull": xTfull,
                "xTloc": np.ascontiguousarray(xTfull[:, sl]),
                "E2T": np.ascontiguousarray(e2b.T),
                "s2loc": np.ascontiguousarray(
                    e2b.astype(np.float32).sum(axis=1)[None, :].astype(bf16)
                ),
                "w_embed": w_embed,
                "w_out": w_out,
                "beta": beta,
            }
        )
    return in_maps


_NC_CACHE = None


def kernel(x, adj, aff_cropping, w_embed, w_out, beta):
    global _NC_CACHE
    _ensure_concourse()
    from concourse.bass_utils import run_bass_kernel_spmd

    if _NC_CACHE is None:
        _NC_CACHE = build_nc()
    nc = _NC_CACHE

    in_maps = make_in_maps(x, adj, aff_cropping, w_embed, w_out, beta)
    res = run_bass_kernel_spmd(nc, in_maps, list(range(NCORES)))
    out = np.concatenate([res.results[r]["out"] for r in range(NCORES)], axis=0)
    fts = np.concatenate([res.results[r]["fts"] for r in range(NCORES)], axis=0)
    return out.astype(np.float32), fts.astype(np.float32)


# revision 21
# speedup vs baseline: 1.1543x; 1.1543x over previous
"""Trainium2 Bass kernel for the A2GNN 2-layer attention GNN (N=8192, d=128).

Row-parallel over 8 NeuronCores: core r owns rows [r*1024, (r+1)*1024).

Math restructuring (verified exact vs the reference on its data distribution):
  h0 = relu(x @ w_embed) >= 0 elementwise, and softmax aggregation keeps
  h >= 0, so cos(h_i, h_j) >= 0 always and the (cos < 0) mask never fires.
  With NEG = -1e9, exp(mask) collapses to a multiplicative factor:
      E1 = E2 * exp(beta*cos),   E2 = aff * (1 + (e^10 - 1)*adj)
  (layer 2 has beta = 0, so P2 = E2 / rowsum(E2) -- no score matmul at all).

Distribution choices (host-side shard prep):
  - E2^T is precomputed per-shard in bf16 (exact: entries are {0,1,e^10})
    and streamed straight into a resident SBUF tensor, read by both layers.
  - x^T is replicated in bf16 so every core embeds the FULL h0 locally --
    no first AllGather; the only collective is the tiny h1 gather.

Everything runs in the transposed [j, i] orientation: scores^T from
qT/qTloc matmuls (q built by a transposed embed h0^T = w^T x^T with norms
from an ones-column matmul and WIDE broadcast-then-reciprocal -- [1,N]
single-lane vector ops are ~6x slower than [128,N] ones), aggregation as
lhsT = h-tile, rhs = E1^T with N=512 moving tensors, rowsums from M=1 ones
matmuls.  Layer-2's s2 = colsum(E2^T) matmuls are issued right after the
AllGather trigger so the PE fills the collective latency; out = h2 @ w_out
is computed at the very end from h2^T (no z-carry through the gather).
The embed chunks and layer-1 j-tiles run as one software pipeline.
"""

import sys

import numpy as np

N = 8192
NCORES = 8
R = N // NCORES          # 1024 rows per core
D = 128                  # hidden dim
NF = 512                 # input features
NCLS = 21                # classes
NJ = N // 128            # 64 j-tiles of 128
NCH = N // 512           # 16 embed chunks of 512
KF = NF // 128           # 4 k-tiles for the embed matmul
KEXP = float(np.exp(10.0) - 1.0)


def _ensure_concourse():
    try:
        import concourse.bass  # noqa: F401
    except ImportError:
        sys.path.insert(0, "/opt/trn_rl_repo")


def _fix_sem_waits(nc, mybir, max_waits=1):
    """This container's walrus accepts at most 1 sem-wait per instruction.
    Keep the first wait on each instruction; move the rest onto preceding
    same-engine NoOps (the engine stalls there first, so semantics are
    preserved).  Ctrl-type instructions (Drain/NoOp) get all waits moved."""
    n_fixed = 0
    for bb in nc.main_func.blocks:
        insts = bb.instructions
        if not any(
            i.sync_info is not None
            and i.sync_info.on_wait
            and len(i.sync_info.on_wait) > max_waits
            for i in insts
        ):
            continue
        out = []
        for ins in insts:
            si = ins.sync_info
            if si is not None and si.on_wait and len(si.on_wait) > max_waits:
                waits = list(si.on_wait)
                is_ctrl = type(ins).__name__ in ("InstDrain", "InstNoOp")
                keep = [] if is_ctrl else waits[:max_waits]
                spill = waits if is_ctrl else waits[max_waits:]
                for k in range(0, len(spill), max_waits):
                    out.append(
                        mybir.InstNoOp(
                            name=f"{ins.name}-dw{k}",
                            engine=ins.engine,
                            bass_nofuse=True,
                            sync_info=mybir.SyncInfo(
                                on_wait=spill[k : k + max_waits], on_update=[]
                            ),
                        )
                    )
                ins.sync_info = mybir.SyncInfo(on_wait=keep, on_update=list(si.on_update))
                n_fixed += 1
            out.append(ins)
        insts.clear()
        insts.extend(out)
    return n_fixed


def build_nc():
    _ensure_concourse()
    import concourse.bass as bass
    import concourse.mybir as mybir
    import concourse.tile as tile
    from concourse import masks

    F32 = mybir.dt.float32
    BF16 = mybir.dt.bfloat16
    AF = mybir.ActivationFunctionType
    core_ids = list(range(NCORES))

    nc = bass.Bass()
    xtf_ext = nc.declare_dram_parameter("xTfull", [NF, N], BF16, isOutput=False)
    xtl_ext = nc.declare_dram_parameter("xTloc", [NF, R], BF16, isOutput=False)
    e2t_ext = nc.declare_dram_parameter("E2T", [N, R], BF16, isOutput=False)
    s2_ext = nc.declare_dram_parameter("s2loc", [1, R], BF16, isOutput=False)
    wemb_ext = nc.declare_dram_parameter("w_embed", [NF, D], F32, isOutput=False)
    wout_ext = nc.declare_dram_parameter("w_out", [D, NCLS], F32, isOutput=False)
    beta_ext = nc.declare_dram_parameter("beta", [1], F32, isOutput=False)
    fts_ext = nc.declare_dram_parameter("fts", [R, D], F32, isOutput=True)
    out_ext = nc.declare_dram_parameter("out", [R, NCLS], F32, isOutput=True)

    with tile.TileContext(nc) as tc:
        with (
            tc.tile_pool(name="persist", bufs=1) as persist,
            tc.tile_pool(name="dram", bufs=1, space="DRAM") as dram,
            tc.tile_pool(name="srs_ps", bufs=2, space="PSUM") as srs_ps,
        ):
            E2T = persist.tile([128, NJ, R], BF16)       # resident masked-exp^T
            qTlocb = persist.tile([128, R], BF16)        # beta * q^T local slice
            wemb_b = persist.tile([128, KF, D], BF16)
            wout_sb = persist.tile([128, NCLS], BF16)
            betab = persist.tile([128, 1], F32)
            ident = persist.tile([128, 128], F32)
            identb = persist.tile([128, 128], BF16)
            ones1 = persist.tile([1, 128], F32)          # K=1 broadcast weights
            ones1b = persist.tile([1, 128], BF16)
            onescolb = persist.tile([128, 1], BF16)      # M=1 column-sum weights

            masks.make_identity(nc, ident[:])
            masks.make_identity(nc, identb[:])
            nc.vector.memset(ones1[:], 1.0)
            nc.vector.memset(ones1b[:], 1.0)
            nc.vector.memset(onescolb[:], 1.0)

            b2_in = dram.tile([R, D], BF16)
            ag2 = dram.tile([N, D], BF16, addr_space="Shared")

            with (
                tc.tile_pool(name="ph1", bufs=1) as ph1,
                tc.tile_pool(name="pa", bufs=3) as pa,
                tc.tile_pool(name="pe", bufs=3) as pe,
                tc.tile_pool(name="aux_ps", bufs=2, space="PSUM") as aux_ps,
                tc.tile_pool(name="sc_ps", bufs=2, space="PSUM") as sc_ps,
                tc.tile_pool(name="agg_ps", bufs=2, space="PSUM") as agg_ps,
            ):
                qT = ph1.tile([128, N], BF16)
                haug1 = ph1.tile([128, NJ, D + 1], BF16)
                nc.vector.memset(haug1[:, :, D : D + 1], 1.0)

                # -------- weights / beta --------
                wtmp = pa.tile([128, KF, D], F32, bufs=1)
                nc.sync.dma_start(
                    wtmp[:], wemb_ext[:].rearrange("(k p) d -> p k d", p=128)
                )
                nc.vector.tensor_copy(wemb_b[:], wtmp[:])
                wotmp = pa.tile([128, NCLS], F32, bufs=1)
                nc.sync.dma_start(wotmp[:], wout_ext[:])
                nc.vector.tensor_copy(wout_sb[:], wotmp[:])
                btmp = pa.tile([1, 1], F32, bufs=1)
                nc.sync.dma_start(btmp[:], beta_ext[None, :])
                ps_b = aux_ps.tile([128, 1], F32, tag="aux")
                nc.tensor.matmul(ps_b[:], ones1[:], btmp[:], start=True, stop=True)
                nc.scalar.copy(betab[:], ps_b[:])

                n2T = ph1.tile([128, NJ], F32)       # per-j squared norms
                rinvjb = ph1.tile([128, NJ], F32)    # beta / |h_j| (exp scales)

                def embed_raw(src_ext, c0, dst_qT, dst_off):
                    """dst_qT[:, cols] = relu(w^T x^T) (bf16, unnormalized);
                    returns the bf16 squared tile for norm accumulation."""
                    xk = pa.tile([128, KF, 512], BF16, tag="xk", bufs=2)
                    for kt in range(KF):
                        nc.sync.dma_start(
                            xk[:, kt, :],
                            src_ext[kt * 128 : (kt + 1) * 128, c0 : c0 + 512],
                        )
                    ps_hT = sc_ps.tile([128, 512], F32, tag="sc")
                    for kt in range(KF):
                        nc.tensor.matmul(
                            ps_hT[:],
                            wemb_b[:, kt, :],
                            xk[:, kt, :],
                            start=(kt == 0),
                            stop=(kt == KF - 1),
                        )
                    hslice = dst_qT[:, dst_off : dst_off + 512]
                    nc.scalar.activation(hslice, ps_hT[:], AF.Relu)
                    sq = pa.tile([128, 512], BF16, tag="sqt", bufs=2)
                    nc.vector.tensor_mul(sq[:], hslice, hslice)
                    return sq

                # local slice first (normalized + beta-scaled, unblocks scores)
                rbeta = pa.tile([128, 1], F32, bufs=1)
                nc.vector.reciprocal(rbeta[:], betab[:])
                for ch in range(2):
                    sq = embed_raw(xtl_ext, ch * 512, qTlocb, ch * 512)
                    ps_n = aux_ps.tile([1, 512], F32, tag="aux")
                    nc.tensor.matmul(ps_n[:], onescolb[:], sq[:], start=True, stop=True)
                    nrm = pa.tile([1, 512], F32, tag="nrm")
                    nc.scalar.sqrt(nrm[:], ps_n[:])
                    nc.vector.tensor_scalar_mul(nrm[:], nrm[:], rbeta[0:1, 0:1])
                    nrmb = pa.tile([1, 512], BF16, tag="nrmb")
                    nc.vector.tensor_copy(nrmb[:], nrm[:])
                    ps_bc = aux_ps.tile([128, 512], F32, tag="aux")
                    nc.tensor.matmul(ps_bc[:], ones1b[:], nrmb[:], start=True, stop=True)
                    rcp = pa.tile([128, 512], F32, tag="rcpw", bufs=2)
                    nc.vector.reciprocal(rcp[:], ps_bc[:])
                    rcpb = pa.tile([128, 512], BF16, tag="rcpwb", bufs=2)
                    nc.vector.tensor_copy(rcpb[:], rcp[:])
                    nc.vector.tensor_mul(
                        qTlocb[:, ch * 512 : (ch + 1) * 512],
                        qTlocb[:, ch * 512 : (ch + 1) * 512],
                        rcpb[:],
                    )

                agg1 = [agg_ps.tile([128, 512], F32, tag="agg", name=f"agg1_{i}") for i in range(2)]
                srs1 = [srs_ps.tile([1, 512], F32, tag="srs", name=f"srs1_{i}") for i in range(2)]

                def prep_chunk(ch):
                    """E2T stream + embed + per-j norms + haug1 transposes."""
                    nc.sync.dma_start(
                        E2T[:, ch * 4 : (ch + 1) * 4, :],
                        e2t_ext[ch * 512 : (ch + 1) * 512, :].rearrange(
                            "(t p) i -> p t i", p=128
                        ),
                    )
                    sq = embed_raw(xtf_ext, ch * 512, qT, ch * 512)
                    for q in range(4):
                        jt = ch * 4 + q
                        ps_nj = aux_ps.tile([128, 1], F32, tag="aux")
                        nc.tensor.matmul(
                            ps_nj[:],
                            sq[:, q * 128 : (q + 1) * 128],
                            onescolb[:],
                            start=True,
                            stop=True,
                        )
                        nc.vector.tensor_copy(n2T[:, jt : jt + 1], ps_nj[:])
                        ps_t = aux_ps.tile([128, 128], BF16, tag="aux")
                        nc.tensor.transpose(
                            ps_t[:], qT[:, jt * 128 : (jt + 1) * 128], identb[:]
                        )
                        nc.vector.tensor_copy(haug1[:, jt, 0:D], ps_t[:])

                def layer1_jt(jt):
                    for bq in range(2):
                        ps_sc = sc_ps.tile([128, 512], F32, tag="sc")
                        nc.tensor.matmul(
                            ps_sc[:],
                            qT[:, jt * 128 : (jt + 1) * 128],
                            qTlocb[:, bq * 512 : (bq + 1) * 512],
                            start=True,
                            stop=True,
                        )
                        e1t = pe.tile([128, 512], BF16, tag="e1t", bufs=3)
                        nc.scalar.activation(
                            e1t[:], ps_sc[:], AF.Exp, scale=rinvjb[:, jt : jt + 1]
                        )
                        nc.vector.tensor_mul(
                            e1t[:], E2T[:, jt, bq * 512 : (bq + 1) * 512], e1t[:]
                        )
                        nc.tensor.matmul(
                            agg1[bq][:],
                            haug1[:, jt, 0:D],
                            e1t[:],
                            start=(jt == 0),
                            stop=(jt == NJ - 1),
                        )
                        nc.tensor.matmul(
                            srs1[bq][:],
                            haug1[:, jt, D : D + 1],
                            e1t[:],
                            start=(jt == 0),
                            stop=(jt == NJ - 1),
                        )

                def finish_norms(c0, cn):
                    nrmj = pa.tile([128, cn], F32, tag="nrmj", bufs=2)
                    nc.scalar.sqrt(nrmj[:], n2T[:, c0 : c0 + cn])
                    nc.vector.reciprocal(
                        rinvjb[:, c0 : c0 + cn], nrmj[:]
                    )
                    nc.vector.tensor_scalar_mul(
                        rinvjb[:, c0 : c0 + cn], rinvjb[:, c0 : c0 + cn], betab[:, 0:1]
                    )

                # block pipeline: prep 4 chunks (16 jts), batch the norm
                # sqrt (avoids ACT Sqrt<->Exp table reloads), trail layer-1
                # by one block
                for blk in range(4):
                    for i in range(4):
                        prep_chunk(blk * 4 + i)
                        if blk >= 1:
                            for q in range(4):
                                layer1_jt((blk - 1) * 16 + i * 4 + q)
                    finish_norms(blk * 16, 16)
                for i in range(4):
                    for q in range(4):
                        layer1_jt(48 + i * 4 + q)

                # ----- layer-1 finish: normalize, bounce, AllGather -----
                for bq in range(2):
                    srsb = pe.tile([1, 512], BF16, tag="srsb", bufs=2)
                    nc.scalar.copy(srsb[:], srs1[bq][:])
                    bcs_ps = aux_ps.tile([128, 512], F32, tag="aux")
                    nc.tensor.matmul(bcs_ps[:], ones1b[:], srsb[:], start=True, stop=True)
                    rcp1 = pe.tile([128, 512], F32, tag="rcp1", bufs=1)
                    nc.vector.reciprocal(rcp1[:], bcs_ps[:])
                    h1T = pe.tile([128, 512], BF16, tag="h1T", bufs=1)
                    nc.vector.tensor_mul(h1T[:], agg1[bq][:], rcp1[:])
                    for q in range(4):
                        r0 = (bq * 4 + q) * 128
                        ps_t = aux_ps.tile([128, 128], BF16, tag="aux")
                        nc.tensor.transpose(
                            ps_t[:], h1T[:, q * 128 : (q + 1) * 128], identb[:]
                        )
                        h1row = pe.tile([128, 128], BF16, tag="h1row", bufs=2)
                        nc.vector.tensor_copy(h1row[:], ps_t[:])
                        nc.scalar.dma_start(
                            b2_in[(bq * 4 + q) * 128 : (bq * 4 + q + 1) * 128, :],
                            h1row[:],
                        )

                nc.gpsimd.collective_compute(
                    "AllGather",
                    mybir.AluOpType.bypass,
                    ins=[b2_in[:]],
                    outs=[ag2[:]],
                    replica_groups=[core_ids],
                )

                # s2 = rowsums of E2 come precomputed from the host
                s2sb = persist.tile([1, R], BF16)
                nc.scalar.dma_start(s2sb[:], s2_ext[:])

            # ===== layer 2: h2^T = (E2 @ h1)^T / s2; out = h2 @ w_out =====
            with (
                tc.tile_pool(name="pf", bufs=3) as pf,
                tc.tile_pool(name="sc2_ps", bufs=2, space="PSUM") as sc2_ps,
                tc.tile_pool(name="agg2_ps", bufs=2, space="PSUM") as agg2_ps,
            ):
                haug2 = pf.tile([128, NJ, D], BF16, bufs=1)
                for grp in range(NJ // 4):
                    nc.scalar.dma_start(
                        haug2[:, grp * 4 : (grp + 1) * 4, :],
                        ag2[grp * 512 : (grp + 1) * 512, :].rearrange(
                            "(t p) c -> p t c", p=128
                        ),
                    )

                agg2 = [agg2_ps.tile([128, 512], F32, tag="agg2", name=f"agg2_{i}") for i in range(2)]
                for jt in range(NJ):
                    for bq in range(2):
                        nc.tensor.matmul(
                            agg2[bq][:],
                            haug2[:, jt, :],
                            E2T[:, jt, bq * 512 : (bq + 1) * 512],
                            start=(jt == 0),
                            stop=(jt == NJ - 1),
                        )

                for bq in range(2):
                    bc2_ps = sc2_ps.tile([128, 512], F32, tag="sc2")
                    nc.tensor.matmul(
                        bc2_ps[:], ones1b[:], s2sb[0:1, bq * 512 : (bq + 1) * 512],
                        start=True, stop=True,
                    )
                    rcp2 = pf.tile([128, 512], F32, tag="rcp2", bufs=2)
                    nc.vector.reciprocal(rcp2[:], bc2_ps[:])
                    h2T = pf.tile([128, 512], BF16, tag="h2T", bufs=2)
                    nc.vector.tensor_mul(h2T[:], agg2[bq][:], rcp2[:])
                    for q in range(4):
                        r0 = (bq * 4 + q) * 128
                        ps_ft = sc2_ps.tile([128, 128], BF16, tag="sc2")
                        nc.tensor.transpose(
                            ps_ft[:], h2T[:, q * 128 : (q + 1) * 128], identb[:]
                        )
                        ftsrow = pf.tile([128, 128], F32, tag="ftsrow", bufs=2)
                        nc.scalar.copy(ftsrow[:], ps_ft[:])
                        nc.sync.dma_start(fts_ext[r0 : r0 + 128, :], ftsrow[:])
                        ps_o = sc2_ps.tile([128, NCLS], F32, tag="sc2")
                        nc.tensor.matmul(
                            ps_o[:],
                            h2T[:, q * 128 : (q + 1) * 128],
                            wout_sb[:],
                            start=True,
                            stop=True,
                        )
                        outrow = pf.tile([128, NCLS], F32, tag="outrow", bufs=2)
                        nc.scalar.copy(outrow[:], ps_o[:])
                        nc.sync.dma_start(out_ext[r0 : r0 + 128, :], outrow[:])

    _fix_sem_waits(nc, __import__("concourse.mybir", fromlist=["mybir"]))
    return nc


def make_in_maps(x, adj, aff_cropping, w_embed, w_out, beta):
    import ml_dtypes

    bf16 = ml_dtypes.bfloat16
    x = np.asarray(x, dtype=np.float32)
    adj = np.asarray(adj, dtype=np.float32)
    aff = np.asarray(aff_cropping, dtype=np.float32)
    w_embed = np.ascontiguousarray(np.asarray(w_embed, dtype=np.float32))
    w_out = np.ascontiguousarray(np.asarray(w_out, dtype=np.float32))
    beta = np.ascontiguousarray(np.asarray(beta, dtype=np.float32))

    xTfull = np.ascontiguousarray(x.T.astype(bf16))
    in_maps = []
    for r in range(NCORES):
        sl = slice(r * R, (r + 1) * R)
        e2 = aff[sl] * (1.0 + KEXP * adj[sl])
        e2b = e2.astype(bf16)
        in_maps.append(
            {
                "xTfull": xTfull,
                "xTloc": np.ascontiguousarray(xTfull[:, sl]),
                "E2T": np.ascontiguousarray(e2b.T),
                "s2loc": np.ascontiguousarray(
                    e2b.astype(np.float32).sum(axis=1)[None, :].astype(bf16)
                ),
                "w_embed": w_embed,
                "w_out": w_out,
                "beta": beta,
            }
        )
    return in_maps


_NC_CACHE = None


def kernel(x, adj, aff_cropping, w_embed, w_out, beta):
    global _NC_CACHE
    _ensure_concourse()
    from concourse.bass_utils import run_bass_kernel_spmd

    if _NC_CACHE is None:
        _NC_CACHE = build_nc()
    nc = _NC_CACHE

    in_maps = make_in_maps(x, adj, aff_cropping, w_embed, w_out, beta)
    res = run_bass_kernel_spmd(nc, in_maps, list(range(NCORES)))
    out = np.concatenate([res.results[r]["out"] for r in range(NCORES)], axis=0)
    fts = np.concatenate([res.results[r]["fts"] for r in range(NCORES)], axis=0)
    return out.astype(np.float32), fts.astype(np.float32)
